# revision 42
# baseline (speedup 1.0000x reference)
"""Trainium2 Bass kernel for the BF16Indexer sparse-attention problem.

Computes, for B=1, M=2048, H=32, D=128, N=4096:
    logits = einsum('bmhd,bnd->bmhn', q, k)          (fp32 accum)
    o      = einsum('bmhn,bmh->bmn', relu(logits), w) / sqrt(D)

Sharding: M (query tokens) split across 8 cores; k replicated.

Per-core algorithm (M_loc = 256 rows, mh = M_loc*H = 8192, 64 mh-tiles):

The 242us baseline ran BOTH matmuls through the PE rhs-stream port
(~218us of array ingest: 512 cols/tile for the logits mm1 plus another
512 cols/tile for the block-diagonal head-reduction mm2).  This version
keeps mm1 on the rhs port but feeds mm2 through the PE's *weight* port,
which otherwise sits idle during mm1:

  unit = (group g of mh-tiles, n-window w of 1024 cols, tile t):
  - mm1 (PE):   2x matmul([128,512]) -> psum1 [128=(4m,32h), 1024] fp32
  - drain (ACT and DVE strictly alternating per unit, one fused
    [128,1024] instr): y' = bf16(relu(scale*psum1)).  This is the
    bottleneck-capacity path: all 33.5M logits pass through it at
    ~1.1-1.2 ns/el per engine, ~584 ns/unit combined.
  - mm2 (PE, B-form): for each 128-col slice s of y':
        matmul(out=psum2[:, 4*gt*s+4t:+4], lhsT=y'[:,128s:+128],
               rhs=wblk4[:, 4*tg:+4])
    y' enters as the STATIONARY operand (FWL-pipelined LDWEIGHTS
    overlaps the running matmuls; measured ~30ns per LDW+MM pair vs
    213ns for an rhs-streamed mm2 tile).  The block-column rhs routes
    sum_h w[m,h]*y'[(m,h),n] to output [128=n, 4=m] -- final o values
    (transposed), all 32 heads contracted in one matmul, no
    accumulation chain.
  - per (g,w): psum2 -> SBUF in two halves staggered 2 units apart
    (so the copy never displaces both drain engines at once), each
    half stored by one 3D-AP DMA to o^T rows.

PSUM: psum1 = 3 x [128,1024] (6 banks), psum2 = 2 x [128,512] (2 banks).
Pipeline: drain trails mm1 by D1=1 unit, B-form mm2 by D2=3 units, all
emitted in one flat stream so the PE never idles.  Loop order is
group-outer ([16,16,16,8,8] tiles; the small final groups halve the
exposed tail store) so qT streams in at ~16GB/s instead of all within
the first window.  Output o is produced TRANSPOSED [N, M_loc]; the
host un-transposes (marshalling only).

Measured: 184.8us HW exec (8 cores SPMD), rel err 1.77e-3, vs 243.8us
baseline.  Steady state ~645ns/unit: PE ~87% busy (mm1 426ns + B-form
~240ns), drains ~80%/75% busy; ramp ~7.5us, store-ack tail ~7us.

kernel(**inputs) takes the FULL inputs and returns the FULL
(1, 2048, 4096) fp32 output.
"""

import math
import numpy as np
import ml_dtypes

import concourse.bass as bass
import concourse.mybir as mybir
import concourse.tile as tile
from concourse import bacc
from concourse.bass_utils import run_bass_kernel_spmd

# Problem constants (hardcoded per harness contract)
B, M, H, D, N = 1, 2048, 32, 128, 4096
N_CORES = 8
M_LOC = M // N_CORES              # 256 query rows per core
MH = M_LOC * H                    # 8192
N_TILES = MH // 128               # 64 mh-tiles (4 m's each)
SOFTMAX_SCALE = 1.0 / math.sqrt(float(D))

W_COLS = 1024                     # n-cols per window
N_WINDOWS = N // W_COLS           # 4
N_SLICES = W_COLS // 128          # 8 B-chunks per unit


def build_nc():
    nc = bacc.Bacc("TRN2", target_bir_lowering=False, debug=False)

    bf16 = mybir.dt.bfloat16
    f32 = mybir.dt.float32

    qT_d = nc.dram_tensor("qT", [128, MH], bf16, kind="ExternalInput")
    kT_d = nc.dram_tensor("kT", [128, N], bf16, kind="ExternalInput")
    wblk4_d = nc.dram_tensor("wblk4", [128, N_TILES * 4], bf16,
                             kind="ExternalInput")
    # o is stored TRANSPOSED: [n, m_loc]
    oT_d = nc.dram_tensor("oT", [N, M_LOC], f32, kind="ExternalOutput")

    with tile.TileContext(nc) as tc:
        with (
            tc.tile_pool(name="const", bufs=1) as const_pool,
            tc.tile_pool(name="ypool", bufs=6) as ypool,
            tc.tile_pool(name="psum1", bufs=3, space="PSUM") as psum1,
            tc.tile_pool(name="psum2", bufs=2, space="PSUM") as psum2,
            tc.tile_pool(name="ostage", bufs=2) as ostage,
        ):
            qT = const_pool.tile([128, MH], bf16)
            kT = const_pool.tile([128, N], bf16)
            wblk4 = const_pool.tile([128, N_TILES * 4], bf16)

            # --- initial DMAs, chunked so early tiles unblock quickly ---
            nc.sync.dma_start(kT[:, :512], kT_d[:, :512])
            nc.sync.dma_start(kT[:, 512:1024], kT_d[:, 512:1024])
            nc.scalar.dma_start(wblk4[:], wblk4_d[:])

            # warm the ACT spline tables while DMAs run
            warm = const_pool.tile([128, 1], bf16)
            nc.vector.memset(warm[:], 0)
            nc.scalar.activation(warm[:], warm[:],
                                 mybir.ActivationFunctionType.Relu)

            # warm the PE (HAM un-throttles after ~3.4us of activity)
            wsrc = const_pool.tile([128, 128], bf16)
            nc.vector.memset(wsrc[:], 0)
            wps = psum1.tile([128, 1024], f32, tag="p1", name="warm_ps")
            for _ in range(36):
                nc.tensor.matmul(wps[:, :128], wsrc[:], wsrc[:],
                                 start=True, stop=True)

            def chunked(eng, dst, src, width, edges):
                lo = 0
                for hi in edges:
                    hi = min(hi, width)
                    if hi > lo:
                        eng.dma_start(dst[:, lo:hi], src[:, lo:hi])
                    lo = hi
                if lo < width:
                    eng.dma_start(dst[:, lo:], src[:, lo:])

            # qT group 0 first, then the rest
            for lo, hi in [(0, 128), (128, 512), (512, 1024), (1024, 2048),
                           (2048, 4096), (4096, 6144), (6144, MH)]:
                nc.gpsimd.dma_start(qT[:, lo:hi], qT_d[:, lo:hi])
            nc.sync.dma_start(kT[:, 1024:2048], kT_d[:, 1024:2048])
            nc.sync.dma_start(kT[:, 2048:], kT_d[:, 2048:])

            # --- main pipeline ---
            # groups of mh-tiles; the last two are small so the final
            # output stores (exposed in the tail) are half-size
            GROUPS = [(0, 16), (16, 16), (32, 16), (48, 8), (56, 8)]
            units = [(gi, w, t) for gi in range(len(GROUPS))
                     for w in range(N_WINDOWS)
                     for t in range(GROUPS[gi][1])]
            D1 = 1   # drain trails mm1 by D1 units
            D2 = 3   # B-form mm2 trails mm1 by D2 units

            p1_of = {}   # unit idx -> psum1 tile
            y_of = {}    # unit idx -> y' tile
            p2_of = {}   # (g, w) -> psum2 tile

            def emit_mm1(i):
                g, w, t = units[i]
                tg = GROUPS[g][0] + t
                p1 = psum1.tile([128, 1024], f32, tag="p1",
                                name=f"p1_{i % 3}")
                qT_t = qT[:, bass.ts(tg, 128)]
                for c in range(2):
                    nc.tensor.matmul(
                        p1[:, bass.ts(c, 512)],
                        qT_t,
                        kT[:, bass.ds(w * W_COLS + c * 512, 512)],
                        start=True, stop=True,
                    )
                p1_of[i] = p1

            def emit_drain(j):
                # one fused relu+scale instr over the whole [128,1024] tile
                p1 = p1_of.pop(j)
                y_t = ypool.tile([128, W_COLS], bf16, tag="y",
                                 name=f"y_{j % 6}")
                if j % 2 == 0:
                    nc.scalar.activation(
                        y_t[:], p1[:],
                        mybir.ActivationFunctionType.Relu,
                        scale=SOFTMAX_SCALE,
                    )
                else:
                    nc.vector.tensor_scalar(
                        y_t[:], p1[:], SOFTMAX_SCALE, 0.0,
                        mybir.AluOpType.mult, mybir.AluOpType.max,
                    )
                y_of[j] = y_t

            def emit_mm2(k):
                g, w, t = units[k]
                t0, gt = GROUPS[g]
                tg = t0 + t
                y_t = y_of.pop(k)
                if (g, w) not in p2_of:
                    p2_of[(g, w)] = psum2.tile([128, 512], f32, tag="p2",
                                               name=f"p2_{(g * 4 + w) % 2}")
                p2 = p2_of[(g, w)]
                for s in range(N_SLICES):
                    nc.tensor.matmul(
                        p2[:, bass.ds(4 * gt * s + 4 * t, 4)],
                        y_t[:, bass.ts(s, 128)],
                        wblk4[:, bass.ts(tg, 4)],
                        start=(t == 0 and s == 0),
                        stop=(t == gt - 1 and s == N_SLICES - 1),
                        skip_group_check=True,
                    )
                if t == gt - 1:
                    finish_gw(g, w, p2_of.pop((g, w)), k + D2)

            deferred = {}  # emission idx -> [closure]

            def finish_gw(g, w, p2, cur_i):
                # psum2 -> SBUF in two halves STAGGERED two units apart so
                # the copy lump never hits both drain engines at once;
                # each half gets its own 3D-AP store DMA.
                t0, gt = GROUPS[g]
                cw = 4 * gt            # m-cols per slice block
                tot = cw * N_SLICES    # total psum2 cols used
                ost = ostage.tile([128, 512], f32, tag="ost",
                                  name=f"ost_{(g * 4 + w) % 2}")
                last = (g == len(GROUPS) - 1 and w == N_WINDOWS - 1)
                nsl = N_SLICES // 2

                def store_half(eng_copy, s0, eng_dma):
                    lo = cw * s0
                    hi = lo + cw * nsl
                    if eng_copy is nc.scalar:
                        nc.scalar.copy(ost[:, lo:hi], p2[:, lo:hi])
                    else:
                        nc.vector.tensor_copy(ost[:, lo:hi], p2[:, lo:hi])
                    dst3 = oT_d[bass.ds(w * W_COLS + 128 * s0, 128 * nsl),
                                bass.ds(4 * t0, cw)].rearrange(
                                    "(s p) c -> p s c", s=nsl)
                    src3 = ost[:, lo:hi].rearrange("p (s c) -> p s c", s=nsl)
                    eng_dma.dma_start(dst3, src3)

                store_half(nc.scalar, 0, nc.sync)
                eng_dma_b = nc.scalar if last else nc.sync
                deferred.setdefault(cur_i + 2, []).append(
                    lambda: store_half(nc.scalar, nsl, eng_dma_b))

            n_units = len(units)
            for i in range(n_units):
                emit_mm1(i)
                if i - D1 >= 0:
                    emit_drain(i - D1)
                if i - D2 >= 0:
                    emit_mm2(i - D2)
                for fn in deferred.pop(i, []):
                    fn()
            for j in range(n_units - D1, n_units):
                emit_drain(j)
            for k in range(n_units - D2, n_units):
                emit_mm2(k)
            for i in sorted(deferred):
                for fn in deferred[i]:
                    fn()

    nc.compile()
    return nc


def marshal_core_inputs(q, k, weights, core):
    """Host-side layout marshalling for one core (no arithmetic)."""
    bf16 = ml_dtypes.bfloat16

    q_sh = np.asarray(q[0, core * M_LOC:(core + 1) * M_LOC])    # (m_loc, H, D)
    qT = np.ascontiguousarray(q_sh.reshape(MH, D).T)            # (128, mh)
    kT = np.ascontiguousarray(np.asarray(k[0]).T)               # (128, n)

    w_sh = np.asarray(weights[core * M_LOC:(core + 1) * M_LOC, 0, :])  # (m_loc, H)
    # wblk4[32j + h, 4t + j] = w[4t + j, h]
    wblk4 = np.zeros((128, N_TILES * 4), dtype=bf16)
    w_r = w_sh.reshape(N_TILES, 4, H)                           # (t, j, h)
    for j in range(4):
        for h in range(H):
            wblk4[32 * j + h, 4 * np.arange(N_TILES) + j] = w_r[:, j, h]

    return {"qT": qT, "kT": kT, "wblk4": wblk4}


_NC_CACHE = {}


def _get_nc():
    if "nc" not in _NC_CACHE:
        _NC_CACHE["nc"] = build_nc()
    return _NC_CACHE["nc"]


def run(inputs, trace=False):
    nc = _get_nc()
    in_maps = [marshal_core_inputs(inputs["q"], inputs["k"], inputs["weights"], c)
               for c in range(N_CORES)]
    res = run_bass_kernel_spmd(nc, in_maps, list(range(N_CORES)), trace=trace)
    # oT is [N, M_LOC] per core -> transpose and concat along m
    out = np.concatenate(
        [np.ascontiguousarray(res.results[c]["oT"].T) for c in range(N_CORES)],
        axis=0)
    return out[None], res  # (1, M, N) fp32


def kernel(q, k, weights):
    out, _ = run({"q": q, "k": k, "weights": weights})
    return out
